# revision 58
# baseline (speedup 1.0000x reference)
"""Trainium2 Bass kernel for nn_Attention_78151224918608.

Dense transformer attention block: QKV proj + RoPE + GQA causal attention
+ output proj. Sharding: batch x head-group over 8 cores: core c handles
batch c//4 and q-heads [8(c%4), 8(c%4)+8) (kv heads [2(c%4), 2(c%4)+2)).
Each core computes a partial output [S, D] (its heads through wo rows);
host sums the 4 partials per batch in fp32 and casts to bf16.

Precision strategy (rel-err budget 2e-2):
  - Scores (Q.K^T) always bf16; the first q-tile (rows 0..511, where
    causal attention concentrates) is full bf16 end-to-end.
  - Projections: token-tile 0 bf16, token-tiles 1-3 fp8e4m3 DoubleRow
    (x/wqkv pre-quantized on host; 2 contraction k-tiles per matmul at
    0.5 cycles/row).
  - P.V: q-tiles 1-3 fp8 DoubleRow (exp writes fp8e4 with a ln(1/16)
    bias folded in; V cast to fp8 on Pool); q-tile 0 bf16.
  - wo: q-tiles 1-3 fp8 DoubleRow on OT*4 (scaled back by 0.25 in the
    evacuation); q-tile 0 bf16.
  - Causal diag masking on PE: fp8 T128^T @ I128 accumulates -240 into
    the partial 128x128 PSUM blocks before exp; the fully-invalid prefix
    of each odd diagonal subtile gets a -240 full-mask matmul so one
    merged exp per pair writes zeros there (no memsets, fewer Act
    instructions); score matmul columns are restricted to the causal
    region.
  - Normalization: recip (DVE) -> partition_broadcast (Pool) ->
    scalar_tensor_tensor (DVE) folding the sumexp recip and the x4 fp8
    scale into the O^T evacuation.
"""

import sys

sys.path.insert(0, "/opt/trn_rl_repo")

import math
import numpy as np
import ml_dtypes

BF16 = ml_dtypes.bfloat16
F8 = ml_dtypes.float8_e4m3

# Problem constants (hardcoded per contract).
B = 2
S = 2048
D = 2048
N_HEADS = 32
N_KV_HEADS = 8
HD = 64
N_CORES = 8
GROUPS = 4  # head-groups per batch
HQ = N_HEADS // GROUPS  # 8 q heads per core
HKV = N_KV_HEADS // GROUPS  # 2 kv heads per core
M_PROJ = (HQ + 2 * HKV) * HD  # 768: [Q0..Q3 | K | V] in 128-blocks
N_MB = M_PROJ // 128  # 6 m-blocks
QTS = 512  # q tile size
KTS = 128  # k tile size
N_QT = S // QTS  # 4
N_DKT = D // 128  # 16 proj contraction tiles
N_SKT = S // KTS  # 16
N_WOK = HQ * HD // 128  # 4 wo contraction tiles


def build_program(phase_log=None):
    import concourse.mybir as mybir
    import concourse.tile as tile
    from concourse import bacc

    def mark(label):
        if phase_log is not None:
            phase_log.append((label, len(nc.inst_map)))

    f32 = mybir.dt.float32
    bf16 = mybir.dt.bfloat16
    fp8 = mybir.dt.float8e4
    Exp = mybir.ActivationFunctionType.Exp
    Copy = mybir.ActivationFunctionType.Copy
    add_op = mybir.AluOpType.add
    mult_op = mybir.AluOpType.mult
    DR = mybir.MatmulPerfMode.DoubleRow

    nc = bacc.Bacc("TRN2", num_devices=N_CORES)
    xT16_d = nc.declare_dram_parameter("xT16", [D, QTS], bf16, isOutput=False)
    xT8_d = nc.declare_dram_parameter("xT8", [D, S - QTS], fp8, isOutput=False)
    wqkv16_d = nc.declare_dram_parameter("wqkv16", [D, M_PROJ], bf16, isOutput=False)
    wqkv8_d = nc.declare_dram_parameter("wqkv8", [D, M_PROJ], fp8, isOutput=False)
    wo16_d = nc.declare_dram_parameter("wo16", [HQ * HD, D], bf16, isOutput=False)
    wo8_d = nc.declare_dram_parameter("wo8", [HQ * HD, D], fp8, isOutput=False)
    cos_d = nc.declare_dram_parameter("cosb", [128, S], bf16, isOutput=False)
    sin_d = nc.declare_dram_parameter("sinb", [128, S], bf16, isOutput=False)
    tmask_d = nc.declare_dram_parameter("tmask", [128, 128], fp8, isOutput=False)
    ident_d = nc.declare_dram_parameter("ident", [128, 128], fp8, isOutput=False)
    tmaskf_d = nc.declare_dram_parameter("tmaskf", [128, 128], fp8, isOutput=False)
    part_d = nc.declare_dram_parameter("part", [S, D], bf16, isOutput=True)

    with tile.TileContext(nc) as tc:
        with (
            tc.tile_pool(name="const", bufs=1) as cpool,
            tc.tile_pool(name="work", bufs=3) as wpool,
            tc.tile_pool(name="estrip", bufs=8) as epool,
            tc.tile_pool(name="outp", bufs=2) as opool,
            tc.tile_pool(name="psw", bufs=2, space="PSUM") as psw,
            tc.tile_pool(name="pssc", bufs=2, space="PSUM") as pssc,
            tc.tile_pool(name="psops", bufs=2, space="PSUM") as psops,
        ):
            # ---- constants / weights / inputs ----
            cos_sb = cpool.tile([128, S], bf16)
            sin_sb = cpool.tile([128, S], bf16)
            tmask_sb = cpool.tile([128, 128], fp8)
            ident_sb = cpool.tile([128, 128], fp8)
            tmaskf_sb = cpool.tile([128, 128], fp8)
            bias_sb = cpool.tile([128, 1], f32)
            wqkv16_sb = cpool.tile([128, N_DKT, M_PROJ], bf16)
            wqkv8_sb = cpool.tile([128, N_DKT, M_PROJ], fp8)
            wo16_sb = cpool.tile([128, N_WOK, D], bf16)
            wo8_sb = cpool.tile([128, N_WOK, D], fp8)
            xT16_sb = cpool.tile([128, N_DKT, QTS], bf16)
            xT8_sb = cpool.tile([128, N_DKT, S - QTS], fp8)

            QT_sb = cpool.tile([128, HQ // 2, S], bf16)  # head pairs stacked
            # K^T per kv head, duplicated across both 64-row halves so the
            # scores lhsT base partition matches qh's (0 or 64).
            KT_sb = [cpool.tile([128, S], bf16, name=f"KT{v}") for v in range(HKV)]
            VT_sb = cpool.tile([128, S], bf16)
            Vb_sb = [
                [cpool.tile([128, 4, 80], bf16, name=f"Vb{v}_{n}") for n in range(N_QT)]
                for v in range(HKV)
            ]
            V8_sb = [
                [cpool.tile([128, 4, 80], fp8, name=f"V8{v}_{n}") for n in range(N_QT)]
                for v in range(HKV)
            ]
            OT16_sb = cpool.tile([128, N_WOK, QTS], bf16)  # qt0
            OT8_sb = cpool.tile([128, N_WOK, S - QTS], fp8)  # qt1..3, x4 scale

            # exp scale 1/16: keeps exp(s_max ~ 7.6) under fp8e4m3 max 448
            nc.gpsimd.memset(bias_sb[:], float(np.log(1.0 / 16.0)))
            for kv in range(HKV):
                for n in range(N_QT):
                    nc.gpsimd.memset(Vb_sb[kv][n][:, :, 64:65], 1.0)
                    nc.gpsimd.memset(V8_sb[kv][n][:, :, 64:65], 1.0)

            # One merged DMA per tensor: the SP sequencer dispatches each
            # dma_start in ~650ns, so instruction count (not bandwidth) was
            # gating the load phase. Transfers run on parallel DMA engines.
            # bf16 proj inputs stream in 4-strip chunks (overlaps proj0);
            # fp8 x arrives per token-tile so proj(n) unblocks incrementally.
            for c in range(4):
                rsl = slice(c * 512, (c + 1) * 512)
                nc.sync.dma_start(
                    wqkv16_sb[:, 4 * c : 4 * c + 4, :],
                    wqkv16_d[rsl, :].rearrange("(k p) c -> p k c", p=128),
                )
                nc.sync.dma_start(
                    xT16_sb[:, 4 * c : 4 * c + 4, :],
                    xT16_d[rsl, :].rearrange("(k p) c -> p k c", p=128),
                )
            nc.sync.dma_start(cos_sb[:, 0:QTS], cos_d[:, 0:QTS])
            nc.sync.dma_start(sin_sb[:, 0:QTS], sin_d[:, 0:QTS])
            nc.sync.dma_start(tmask_sb[:], tmask_d[:])
            nc.sync.dma_start(ident_sb[:], ident_d[:])
            nc.sync.dma_start(tmaskf_sb[:], tmaskf_d[:])
            nc.sync.dma_start(
                wqkv8_sb[:], wqkv8_d[:].rearrange("(k p) c -> p k c", p=128)
            )
            for n in range(1, N_QT):
                csl = slice((n - 1) * QTS, n * QTS)
                nc.sync.dma_start(
                    xT8_sb[:, :, csl],
                    xT8_d[:, csl].rearrange("(k p) c -> p k c", p=128),
                )
            nc.sync.dma_start(cos_sb[:, QTS:], cos_d[:, QTS:])
            nc.sync.dma_start(sin_sb[:, QTS:], sin_d[:, QTS:])
            nc.sync.dma_start(
                wo16_sb[:], wo16_d[:].rearrange("(k p) c -> p k c", p=128)
            )
            nc.sync.dma_start(
                wo8_sb[:], wo8_d[:].rearrange("(k p) c -> p k c", p=128)
            )

            # ---------------- projection ----------------

            def rope_block(ps, m, nsl, on_act=False):
                """Evacuate proj PSUM block m (token cols nsl) via rope.
                PSUM->SBUF copies run on Act (idle during proj); rope math on
                DVE with partition-shifted reads replacing the swap copies."""
                if m == N_MB - 1:  # V block: plain copy
                    if on_act:
                        nc.scalar.activation(VT_sb[:, nsl], ps[:], Copy)
                    else:
                        nc.vector.tensor_copy(VT_sb[:, nsl], ps[:])
                    return
                q_raw = wpool.tile([128, QTS], bf16, tag="qraw")
                t1 = wpool.tile([128, QTS], bf16, tag="t1")
                t2 = wpool.tile([128, QTS], bf16, tag="t2")
                if on_act:
                    nc.scalar.activation(q_raw[:], ps[:], Copy)
                else:
                    nc.vector.tensor_copy(q_raw[:], ps[:])
                nc.vector.tensor_tensor(t1[:], q_raw[:], cos_sb[:, nsl], mult_op)
                for r0, r1 in ((0, 32), (32, 0), (64, 96), (96, 64)):
                    nc.vector.tensor_copy(t2[r0 : r0 + 32, :], q_raw[r1 : r1 + 32, :])
                nc.vector.tensor_tensor(t2[:], t2[:], sin_sb[:, nsl], mult_op)
                if m < HQ // 2:
                    nc.vector.tensor_tensor(QT_sb[:, m, nsl], t1[:], t2[:], add_op)
                else:
                    # K block rows: [kv0 (0:64) | kv1 (64:128)]; write each kv
                    # into its own tile and duplicate across both halves.
                    nc.vector.tensor_tensor(
                        KT_sb[0][0:64, nsl], t1[0:64, :], t2[0:64, :], add_op
                    )
                    nc.vector.tensor_tensor(
                        KT_sb[1][64:128, nsl], t1[64:128, :], t2[64:128, :], add_op
                    )
                    nc.vector.tensor_copy(KT_sb[0][64:128, nsl], KT_sb[0][0:64, nsl])
                    nc.vector.tensor_copy(KT_sb[1][0:64, nsl], KT_sb[1][64:128, nsl])

            def proj_block(n, m):
                nsl = slice(n * QTS, (n + 1) * QTS)
                msl = slice(m * 128, (m + 1) * 128)
                ps = psw.tile([128, QTS], f32, tag="w")
                if n == 0:
                    for kt in range(N_DKT):
                        nc.tensor.matmul(
                            ps[:],
                            wqkv16_sb[:, kt, msl],
                            xT16_sb[:, kt, :],
                            start=(kt == 0),
                            stop=(kt == N_DKT - 1),
                        )
                else:
                    nsl8 = slice((n - 1) * QTS, n * QTS)
                    for t in range(N_DKT // 2):
                        nc.tensor.matmul(
                            ps[:],
                            wqkv8_sb[:, 2 * t : 2 * t + 2, msl],
                            xT8_sb[:, 2 * t : 2 * t + 2, nsl8],
                            start=(t == 0),
                            stop=(t == N_DKT // 2 - 1),
                            perf_mode=DR,
                        )
                rope_block(ps, m, nsl, on_act=(n == 0))

            def v_tx(n):
                # V^T -> token-major V (+ fp8 cast) for token tile n; one
                # merged 4-tile transpose + one cast per kv head
                nsl = slice(n * QTS, (n + 1) * QTS)
                for kv in range(HKV):
                    nc.sync.dma_start_transpose(
                        Vb_sb[kv][n][:, :, 0:64], VT_sb[64 * kv : 64 * kv + 64, nsl]
                    )
                    nc.gpsimd.tensor_copy(
                        V8_sb[kv][n][:, :, 0:64], Vb_sb[kv][n][:, :, 0:64]
                    )

            def proj_units(n):
                # KV blocks first so attention on this token tile unblocks early
                units = [lambda m=m: proj_block(n, m) for m in (4, 5)]
                units.append(lambda: v_tx(n))
                units += [lambda m=m: proj_block(n, m) for m in range(HQ // 2)]
                return units

            # ---------------- attention ----------------
            pending = []

            def normalize():
                h, qt, ops, rt = pending.pop(0)
                hb = 64 * (h % 2)
                qblk = h // 2
                bsb = wpool.tile([64, QTS], f32, tag="bsb")
                nc.gpsimd.partition_broadcast(bsb[:], rt[:])
                dst = (
                    OT16_sb[hb : hb + 64, qblk, :]
                    if qt == 0
                    else OT8_sb[hb : hb + 64, qblk, (qt - 1) * QTS : qt * QTS]
                )
                scale = 1.0 if qt == 0 else 4.0
                nc.vector.scalar_tensor_tensor(
                    dst, ops[0:64, :], scale, bsb[:], mult_op, mult_op
                )

            def attn(qt, fillers=(), pop_before=False):
                mark(f"attn{qt}")
                fillers = list(fillers)
                while pending:
                    normalize()
                qsl = slice(qt * QTS, (qt + 1) * QTS)
                n_kt = 4 * (qt + 1)
                for h in range(HQ):
                    if pop_before:
                        while fillers and len(fillers) > HQ - h - 1:
                            fillers.pop(0)()
                        if fillers:
                            fillers.pop(0)()
                    kv = h // 4
                    hb = 64 * (h % 2)
                    qh = QT_sb[hb : hb + 64, h // 2, qsl]
                    kt2 = KT_sb[kv][hb : hb + 64, :]
                    ops = psops.tile([128, QTS], f32, tag="ops")
                    for g in range(0, n_kt, 2):
                        diag = g >= n_kt - 4
                        sc = pssc.tile([128, 2, QTS], f32, tag="sc")
                        for j in range(2):
                            kt = g + j
                            o = kt * KTS - qt * QTS
                            if o < 0:  # full tile
                                nc.tensor.matmul(
                                    sc[:, j, :],
                                    kt2[:, kt * KTS : (kt + 1) * KTS],
                                    qh[:],
                                    start=True,
                                    stop=True,
                                )
                            else:  # diagonal tile: restrict + mask add
                                if j == 1:
                                    # odd subtile: fully-invalid prefix gets
                                    # -240 so the merged exp writes zeros
                                    nc.tensor.matmul(
                                        sc[:, j, o - 128 : o],
                                        tmaskf_sb[:],
                                        ident_sb[:],
                                        start=True,
                                        stop=True,
                                    )
                                nc.tensor.matmul(
                                    sc[:, j, o:QTS],
                                    kt2[:, kt * KTS : (kt + 1) * KTS],
                                    qh[:, o:QTS],
                                    start=True,
                                    stop=False,
                                )
                                nc.tensor.matmul(
                                    sc[:, j, o : o + 128],
                                    tmask_sb[:],
                                    ident_sb[:],
                                    start=False,
                                    stop=True,
                                )
                        if g == 0 and pending:
                            normalize()
                        o0 = g * KTS - qt * QTS  # col offset of even subtile
                        if qt == 0:
                            # bf16 path: one merged exp + per-tile PV
                            e = epool.tile([128, 2, QTS], bf16, tag="e16")
                            nc.scalar.activation(
                                e[:, :, o0:QTS], sc[:, :, o0:QTS], Exp,
                                bias=bias_sb[:],
                            )
                            for j in range(2):
                                kt = g + j
                                o = o0 + j * KTS
                                nc.tensor.matmul(
                                    ops[0:65, o:QTS],
                                    Vb_sb[kv][kt // 4][:, kt % 4, 0:65],
                                    e[:, j, o:QTS],
                                    start=(kt == 0),
                                    stop=(kt == n_kt - 1),
                                )
                        else:
                            e = epool.tile([128, 2, QTS], fp8, tag="e8")
                            if not diag:
                                nc.scalar.activation(
                                    e[:, :, :], sc[:, :, :], Exp, bias=bias_sb[:]
                                )
                                cols = slice(0, QTS)
                            else:
                                nc.scalar.activation(
                                    e[:, :, o0:QTS], sc[:, :, o0:QTS], Exp,
                                    bias=bias_sb[:],
                                )
                                cols = slice(o0, QTS)
                            nc.tensor.matmul(
                                ops[0:65, cols],
                                V8_sb[kv][g // 4][:, g % 4 : g % 4 + 2, 0:65],
                                e[:, :, cols],
                                start=(g == 0),
                                stop=(g == n_kt - 2),
                                perf_mode=DR,
                            )
                    # eager reciprocal; broadcast+scale deferred via pending
                    rt = wpool.tile([1, QTS], f32, tag="rt")
                    nc.vector.reciprocal(rt[:], ops[64:65, :])
                    pending.append((h, qt, ops, rt))
                    if not pop_before and fillers:
                        fillers.pop(0)()
                for f in fillers:
                    f()

            # ---------------- output projection ----------------

            def wo_mt(qt, mt):
                tl = (mt % 4) * 128
                osb = opool.tile([128, D], bf16, tag="osb")
                for nn in range(D // QTS):
                    nsl = slice(nn * QTS, (nn + 1) * QTS)
                    ps = psw.tile([128, QTS], f32, tag="w")
                    if qt == 0:
                        for kt in range(N_WOK):
                            nc.tensor.matmul(
                                ps[:],
                                OT16_sb[:, kt, tl : tl + 128],
                                wo16_sb[:, kt, nsl],
                                start=(kt == 0),
                                stop=(kt == N_WOK - 1),
                            )
                        nc.vector.tensor_copy(osb[:, nsl], ps[:])
                        nc.sync.dma_start(
                            part_d[mt * 128 : (mt + 1) * 128, nsl], osb[:, nsl]
                        )
                    else:
                        col0 = (qt - 1) * QTS + tl
                        for t in range(N_WOK // 2):
                            nc.tensor.matmul(
                                ps[:],
                                OT8_sb[:, 2 * t : 2 * t + 2, col0 : col0 + 128],
                                wo8_sb[:, 2 * t : 2 * t + 2, nsl],
                                start=(t == 0),
                                stop=(t == N_WOK // 2 - 1),
                                perf_mode=DR,
                            )
                        # undo OT8 x4 and wo8 x16 host pre-scales
                        if qt == 3 and nn % 2 == 0:
                            nc.scalar.activation(
                                osb[:, nsl], ps[:], Copy, scale=0.25 / 16.0
                            )
                        else:
                            nc.vector.tensor_scalar_mul(
                                osb[:, nsl], ps[:], 0.25 / 16.0
                            )
                nc.sync.dma_start(part_d[mt * 128 : (mt + 1) * 128, :], osb[:])

            def wo_units(qt):
                return [lambda mt=mt: wo_mt(qt, mt) for mt in range(4 * qt, 4 * qt + 4)]

            def proj_kv(n):
                return [
                    lambda: proj_block(n, 4),
                    lambda: proj_block(n, 5),
                    lambda: v_tx(n),
                ]

            def proj_q(n):
                return [lambda m=m: proj_block(n, m) for m in range(HQ // 2)]

            mark("proj0")
            for u in proj_units(0):
                u()
            attn(0, proj_q(1) + proj_kv(1), pop_before=True)
            attn(1, proj_q(2) + proj_kv(2), pop_before=True)
            attn(2, proj_q(3) + proj_kv(3), pop_before=True)
            attn(3, wo_units(0) + wo_units(1) + wo_units(2), pop_before=True)
            mark("wo3")
            while pending:
                normalize()
            for u in wo_units(3):
                u()
    mark("end")
    nc.compile()
    return nc


# ---------------- host-side sharding ----------------

_PERM = np.concatenate([np.arange(0, HD, 2), np.arange(1, HD, 2)])  # evens, odds


def make_core_inputs(x, freqs_cos, freqs_sin, wq, wk, wv, wo):
    """Build per-core input maps (list of dicts, one per core)."""
    # fp8 pre-scaling: weights x16, x /16 (product scale cancels). Keeps both
    # operands out of the fp8e4m3 subnormal range (min normal 2^-6).
    FS = 16.0
    xT = [np.ascontiguousarray(x[b].T) for b in range(B)]  # [D, S] f32
    xT16 = [t[:, 0:QTS].astype(BF16) for t in xT]
    xT8 = [np.ascontiguousarray(t[:, QTS:] / FS).astype(F8) for t in xT]

    cosT = np.ascontiguousarray(freqs_cos.T)  # [32, S]
    sinT = np.ascontiguousarray(freqs_sin.T)
    cosb = np.tile(np.concatenate([cosT, cosT], axis=0), (2, 1)).astype(BF16)
    sinb = np.tile(np.concatenate([-sinT, sinT], axis=0), (2, 1)).astype(BF16)

    k = np.arange(128)[:, None]
    p = np.arange(128)[None, :]
    tmask = np.where(p > k, -240.0, 0.0).astype(F8)  # [k, p]
    tmaskf = np.full((128, 128), -240.0).astype(F8)
    ident = np.eye(128).astype(F8)

    scale = 1.0 / math.sqrt(HD)
    in_maps = []
    for c in range(N_CORES):
        b, g = c // GROUPS, c % GROUPS
        heads = range(HQ * g, HQ * g + HQ)
        kvs = range(HKV * g, HKV * g + HKV)
        wq_c = np.concatenate(
            [wq[:, h * HD : (h + 1) * HD][:, _PERM] for h in heads], axis=1
        ) * scale
        wk_c = np.concatenate(
            [wk[:, v * HD : (v + 1) * HD][:, _PERM] for v in kvs], axis=1
        )
        wv_c = np.concatenate(
            [wv[:, v * HD : (v + 1) * HD] for v in kvs], axis=1
        )
        wqkv = np.concatenate([wq_c, wk_c, wv_c], axis=1)  # [D, 768] f32
        wo_c = np.ascontiguousarray(
            wo[HQ * g * HD : (HQ * g + HQ) * HD, :]
        )  # [512, D]
        in_maps.append(
            {
                "xT16": xT16[b],
                "xT8": xT8[b],
                "wqkv16": wqkv.astype(BF16),
                "wqkv8": (wqkv * FS).astype(F8),
                "wo16": wo_c.astype(BF16),
                "wo8": (wo_c * FS).astype(F8),
                "cosb": cosb,
                "sinb": sinb,
                "tmask": tmask,
                "tmaskf": tmaskf,
                "ident": ident,
            }
        )
    return in_maps


_NC_CACHE = {}


def kernel(x, freqs_cos, freqs_sin, wq, wk, wv, wo):
    from concourse.bass_utils import run_bass_kernel_spmd

    x = np.asarray(x, np.float32)
    freqs_cos = np.asarray(freqs_cos, np.float32)
    freqs_sin = np.asarray(freqs_sin, np.float32)
    wq = np.asarray(wq, np.float32)
    wk = np.asarray(wk, np.float32)
    wv = np.asarray(wv, np.float32)
    wo = np.asarray(wo, np.float32)

    if "nc" not in _NC_CACHE:
        _NC_CACHE["nc"] = build_program()
    nc = _NC_CACHE["nc"]

    in_maps = make_core_inputs(x, freqs_cos, freqs_sin, wq, wk, wv, wo)
    res = run_bass_kernel_spmd(nc, in_maps, list(range(N_CORES)))
    out = np.zeros((B, S, D), np.float32)
    for c in range(N_CORES):
        out[c // GROUPS] += np.asarray(res.results[c]["part"], np.float32)
    return out.astype(BF16)
